# revision 4
# baseline (speedup 1.0000x reference)
"""GRU cell kernel for Trainium2, data-parallel over 8 NeuronCores.

Computation (per reference):
    gx[g] = x @ wx[g] + bx[g]
    gh[g] = hid @ wh[g] + bh[g]
    r = sigmoid(gx0 + gh0); z = sigmoid(gx1 + gh1)
    n = tanh(gx2 + r * gh2)
    out = (1 - z) * n + z * hid

Design (v2):
  - Batch (8192) sharded 8 ways -> 1024 rows/core; weights replicated.
  - out^T computed in [H-partition, B-free] layout; per-partition biases.
  - Mixed precision to cut PE + DMA cost while staying ~10x under the
    2e-2 error gate (measured rel-err 9.6e-3 on the reference data):
      * r-gate: both sides fp8e4m3 (x*16, w*512; dequant 2^-13 in the
        sigmoid's scale) using DoubleRow perf mode -> 2x PE throughput.
      * z/n-gate x-side: bf16 activations + weights.
      * z/n-gate h-side: f32r (exact fp32 bits; 1 cyc/row at moving>=256)
        reusing the f32 h slabs that the final blend needs anyway.
  - reps>1 repeats the whole per-rep body (DMA + compute) for slope
    timing: HW time = (t(reps=R) - t(reps=1)) / (R - 1).
"""

import numpy as np

B, I, H = 8192, 1024, 1024
NCORES = 8
BL = B // NCORES  # 1024 batch rows per core
P = 128           # partitions
KT = I // P       # 8 contraction tiles
MT = H // P       # 8 output H tiles
NB = 512          # moving free dim per matmul
NBT = BL // NB    # 2 batch slices
KP = KT // 2      # 4 fp8 DoubleRow k-pairs

XS = 16.0         # fp8 activation scale
WS = 512.0        # fp8 weight scale
DESCALE = 1.0 / (XS * WS)  # 2^-13, exact

_built = {}  # reps -> nc


def _build(reps=1):
    import concourse.bass as bass
    import concourse.mybir as mybir
    from concourse.bass import ts
    from concourse.tile import TileContext

    dt = mybir.dt
    f32 = dt.float32
    f32r = dt.float32r
    bf16 = dt.bfloat16
    f8 = dt.float8e4
    ACT = mybir.ActivationFunctionType
    ALU = mybir.AluOpType
    DR = mybir.MatmulPerfMode.DoubleRow

    nc = bass.Bass()
    hT = nc.declare_dram_parameter("hT", [H, BL], f32r, isOutput=False)
    xbT = nc.declare_dram_parameter("xbT", [I, BL], bf16, isOutput=False)
    x8 = nc.declare_dram_parameter("x8", [P, KT, BL], f8, isOutput=False)
    h8 = nc.declare_dram_parameter("h8", [P, KT, BL], f8, isOutput=False)
    wzx = nc.declare_dram_parameter("wzx", [MT, P, KT * P], bf16, isOutput=False)
    wnx = nc.declare_dram_parameter("wnx", [MT, P, KT * P], bf16, isOutput=False)
    wzh = nc.declare_dram_parameter("wzh", [MT, P, KT * P], f32r, isOutput=False)
    wnh = nc.declare_dram_parameter("wnh", [MT, P, KT * P], f32r, isOutput=False)
    wr8 = nc.declare_dram_parameter("wr8", [2, MT, P, KT, P], f8, isOutput=False)
    br = nc.declare_dram_parameter("br", [P, MT], f32, isOutput=False)
    bz = nc.declare_dram_parameter("bz", [P, MT], f32, isOutput=False)
    bxn = nc.declare_dram_parameter("bxn", [P, MT], f32, isOutput=False)
    bhn = nc.declare_dram_parameter("bhn", [P, MT], f32, isOutput=False)
    outT = nc.declare_dram_parameter("outT", [H, BL], f32, isOutput=True)

    with TileContext(nc) as tc:
        with (
            tc.tile_pool(name="const", bufs=1) as cpool,
            tc.tile_pool(name="acts", bufs=2 if reps > 1 else 1) as apool,
            tc.tile_pool(name="w", bufs=2) as wpool,
            tc.tile_pool(name="ew", bufs=2) as epool,
            tc.tile_pool(name="ob", bufs=3) as opool,
            tc.tile_pool(name="ps", bufs=2, space="PSUM") as ppool,
        ):
            br_t = cpool.tile([P, MT], f32, tag="br")
            bz_t = cpool.tile([P, MT], f32, tag="bz")
            bxn_t = cpool.tile([P, MT], f32, tag="bxn")
            bhn_t = cpool.tile([P, MT], f32, tag="bhn")
            nc.sync.dma_start(out=br_t[:], in_=br[:])
            nc.sync.dma_start(out=bz_t[:], in_=bz[:])
            nc.sync.dma_start(out=bxn_t[:], in_=bxn[:])
            nc.sync.dma_start(out=bhn_t[:], in_=bhn[:])

            for _rep in range(reps):
                # fp8 r-gate activations, k-pair granular so the PE can
                # start the DoubleRow chains almost immediately.
                x8t = apool.tile([P, KT, BL], f8, tag="x8")
                h8t = apool.tile([P, KT, BL], f8, tag="h8")
                for kk in range(KP):
                    sl = slice(2 * kk, 2 * kk + 2)
                    nc.sync.dma_start(out=x8t[:, sl, :], in_=x8[:, sl, :])
                    nc.sync.dma_start(out=h8t[:, sl, :], in_=h8[:, sl, :])

                # m=0 weights before the slab bulk so the PE starts fast.
                def load_w(m):
                    wt = {
                        "wr8x": (wpool.tile([P, KT, P], f8, tag="wr8x", name="wr8x"), wr8[0, m]),
                        "wr8h": (wpool.tile([P, KT, P], f8, tag="wr8h", name="wr8h"), wr8[1, m]),
                        "wzx": (wpool.tile([P, KT * P], bf16, tag="wzx", name="wzx"), wzx[m]),
                        "wnx": (wpool.tile([P, KT * P], bf16, tag="wnx", name="wnx"), wnx[m]),
                        "wzh": (wpool.tile([P, KT * P], f32r, tag="wzh", name="wzh"), wzh[m]),
                        "wnh": (wpool.tile([P, KT * P], f32r, tag="wnh", name="wnh"), wnh[m]),
                    }
                    for tile, src in wt.values():
                        nc.sync.dma_start(out=tile[:], in_=src)
                    return {k: v[0] for k, v in wt.items()}

                w0 = load_w(0)

                # Resident z/n activations: x bf16 slabs, h f32 slabs
                # (h f32 doubles as the blend input).
                xbk = [apool.tile([P, BL], bf16, tag=f"xb{k}", name=f"xb{k}") for k in range(KT)]
                hk = [apool.tile([P, BL], f32r, tag=f"h{k}", name=f"h{k}") for k in range(KT)]
                for k in range(KT):
                    nc.sync.dma_start(out=xbk[k][:], in_=xbT[ts(k, P), :])
                    nc.sync.dma_start(out=hk[k][:], in_=hT[ts(k, P), :])

                for m in range(MT):
                    w = w0 if m == 0 else load_w(m)
                    for b in range(NBT):
                        bs = ts(b, NB)
                        s_r = ppool.tile([P, NB], f32, tag="s_r")
                        s_z = ppool.tile([P, NB], f32, tag="s_z")
                        g2x = ppool.tile([P, NB], f32, tag="g2x")
                        g2h = ppool.tile([P, NB], f32, tag="g2h")
                        if b == 0:
                            # Dummy 1-column LDWEIGHTS touching each fresh
                            # weight slab: absorbs the DMA-completion wait so
                            # no matmul carries >1 sync wait (walrus codegen
                            # limit for the fused LW+MM lowering).
                            for w6 in w.values():
                                nc.tensor.ldweights(w6.bitcast(bf16)[:, 0:1])
                        # r-gate: fp8 DoubleRow, both sides into one bank.
                        for kk in range(KP):
                            sl = slice(2 * kk, 2 * kk + 2)
                            nc.tensor.matmul(
                                s_r[:], w["wr8x"][:, sl, :], x8t[:, sl, bs],
                                start=kk == 0, stop=False, perf_mode=DR)
                            nc.tensor.matmul(
                                s_r[:], w["wr8h"][:, sl, :], h8t[:, sl, bs],
                                start=False, stop=kk == KP - 1, perf_mode=DR)
                        # z/n gates: bf16 x-side + f32r h-side.
                        for k in range(KT):
                            ks = ts(k, P)
                            xm = xbk[k][:, bs]
                            hm = hk[k][:, bs]
                            first = k == 0
                            last = k == KT - 1
                            nc.tensor.matmul(
                                s_z[:], w["wzx"][:, ks], xm,
                                start=first, stop=False)
                            nc.tensor.matmul(
                                g2x[:], w["wnx"][:, ks], xm,
                                start=first, stop=last)
                            nc.tensor.matmul(
                                s_z[:], w["wzh"][:, ks], hm,
                                start=False, stop=last)
                            nc.tensor.matmul(
                                g2h[:], w["wnh"][:, ks], hm,
                                start=first, stop=last)

                        mcol = slice(m, m + 1)
                        r = epool.tile([P, NB], f32, tag="r")
                        z = epool.tile([P, NB], f32, tag="z")
                        t = epool.tile([P, NB], f32, tag="t")
                        u = epool.tile([P, NB], f32, tag="u")
                        n = epool.tile([P, NB], f32, tag="n")
                        d = epool.tile([P, NB], f32, tag="d")
                        v = epool.tile([P, NB], f32, tag="v")
                        ob = opool.tile([P, NB], f32, tag="ob")
                        # r = sigmoid(s_r * 2^-13 + br), z = sigmoid(s_z + bz)
                        nc.scalar.activation(
                            r[:], s_r[:], ACT.Sigmoid,
                            bias=br_t[:, mcol], scale=DESCALE)
                        nc.scalar.activation(
                            z[:], s_z[:], ACT.Sigmoid, bias=bz_t[:, mcol])
                        # t = (g2h + bhn) * r ; u = (g2x + bxn) + t ; n = tanh(u)
                        nc.vector.scalar_tensor_tensor(
                            t[:], g2h[:], bhn_t[:, mcol], r[:],
                            op0=ALU.add, op1=ALU.mult)
                        nc.vector.scalar_tensor_tensor(
                            u[:], g2x[:], bxn_t[:, mcol], t[:],
                            op0=ALU.add, op1=ALU.add)
                        nc.scalar.activation(n[:], u[:], ACT.Tanh)
                        # out = (hid - n) * z + n
                        nc.vector.tensor_sub(d[:], hk[m][:, bs].bitcast(f32), n[:])
                        nc.vector.tensor_mul(v[:], d[:], z[:])
                        nc.vector.tensor_add(ob[:], v[:], n[:])
                        nc.sync.dma_start(out=outT[ts(m, P), bs], in_=ob[:])

    _split_waits(nc)
    return nc


def _split_waits(nc):
    """Walrus codegen encodes at most one semaphore wait per engine
    instruction. Tile can emit several; split the extras onto InstNoOp
    instructions inserted immediately before (same engine, same order --
    semantically identical to the multi-wait)."""
    import concourse.mybir as mybir

    SKIP = ("InstEventSemaphore", "InstCall", "InstUnconditionalBranch")
    for bb in nc.main_func.blocks:
        insts = list(bb.instructions)
        out = []
        changed = False
        for inst in insts:
            si = inst.sync_info
            nm = type(inst).__name__
            if (si is not None and si.on_wait and len(si.on_wait) > 1
                    and nm not in SKIP):
                waits = list(si.on_wait)
                for w in waits[:-1]:
                    nop = mybir.InstNoOp(
                        name=nc.get_next_instruction_name(),
                        engine=inst.engine, ins=[], outs=[])
                    nop.sync_info = mybir.SyncInfo(on_wait=[w], on_update=[])
                    nc.register_instruction(nop)
                    out.append(nop)
                inst.sync_info = mybir.SyncInfo(
                    on_wait=[waits[-1]], on_update=list(si.on_update or []))
                changed = True
            out.append(inst)
        if changed:
            bb.instructions = out


def _tile_w(w, np_dtype, scale=1.0):
    # [I, H] -> [MT, P, KT*P]: out[m, p, k*128+j] = w[k*128+p, m*128+j]
    w5 = np.asarray(w, np.float32) * scale
    w5 = w5.reshape(KT, P, MT, P).transpose(2, 1, 0, 3)  # m, p, k, j
    return np.ascontiguousarray(w5.reshape(MT, P, KT * P).astype(np_dtype))


def _tile_b(vec):  # [H] -> [P, MT] with [p, m] = vec[m*128+p]
    return np.ascontiguousarray(np.asarray(vec, np.float32).reshape(MT, P).T)


def _prep_shared(wx, wh, bx, bh):
    import ml_dtypes

    bf16 = ml_dtypes.bfloat16
    f8 = ml_dtypes.float8_e4m3
    wx = np.asarray(wx, np.float32)
    wh = np.asarray(wh, np.float32)
    bx = np.asarray(bx, np.float32)
    bh = np.asarray(bh, np.float32)

    wr8 = np.stack([
        _tile_w(wx[0], f8, WS).reshape(MT, P, KT, P),
        _tile_w(wh[0], f8, WS).reshape(MT, P, KT, P),
    ])
    return {
        "wzx": _tile_w(wx[1], bf16),
        "wnx": _tile_w(wx[2], bf16),
        "wzh": _tile_w(wh[1], np.float32),
        "wnh": _tile_w(wh[2], np.float32),
        "wr8": wr8,
        "br": _tile_b(bx[0] + bh[0]),
        "bz": _tile_b(bx[1] + bh[1]),
        "bxn": _tile_b(bx[2]),
        "bhn": _tile_b(bh[2]),
    }


def _prep_core(x_rows, h_rows):
    """Per-core activation tensors from [BL, I]/[BL, H] fp32 row shards."""
    import ml_dtypes

    bf16 = ml_dtypes.bfloat16
    f8 = ml_dtypes.float8_e4m3
    xT = np.ascontiguousarray(x_rows.T)                 # [I, BL] f32
    hT = np.ascontiguousarray(h_rows.T)                 # [H, BL] f32
    x8 = np.ascontiguousarray(
        (xT * XS).astype(f8).reshape(KT, P, BL).transpose(1, 0, 2))
    h8 = np.ascontiguousarray(
        (hT * XS).astype(f8).reshape(KT, P, BL).transpose(1, 0, 2))
    return {
        "hT": hT,
        "xbT": np.ascontiguousarray(xT.astype(bf16)),
        "x8": x8,
        "h8": h8,
    }


def make_in_maps(x, hid, wx, wh, bx, bh):
    x = np.asarray(x, np.float32)
    hid = np.asarray(hid, np.float32)
    shared = _prep_shared(wx, wh, bx, bh)
    in_maps = []
    for c in range(NCORES):
        rows = slice(c * BL, (c + 1) * BL)
        m = dict(shared)
        m.update(_prep_core(x[rows], hid[rows]))
        in_maps.append(m)
    return in_maps


def kernel(x, hid, wx, wh, bx, bh):
    from concourse.bass_utils import run_bass_kernel_spmd

    if 1 not in _built:
        _built[1] = _build(reps=1)
    nc = _built[1]

    in_maps = make_in_maps(x, hid, wx, wh, bx, bh)
    res = run_bass_kernel_spmd(nc, in_maps, list(range(NCORES)))
    out = np.empty((B, H), np.float32)
    for c in range(NCORES):
        out[c * BL:(c + 1) * BL] = res.results[c]["outT"].T
    return out


# revision 10
# speedup vs baseline: 1.0020x; 1.0020x over previous
"""GRU cell kernel for Trainium2, data-parallel over 8 NeuronCores.

Computation (per reference):
    gx[g] = x @ wx[g] + bx[g]
    gh[g] = hid @ wh[g] + bh[g]
    r = sigmoid(gx0 + gh0); z = sigmoid(gx1 + gh1)
    n = tanh(gx2 + r * gh2)
    out = (1 - z) * n + z * hid

Design (v6):
  - Batch (8192) sharded 8 ways -> 1024 rows/core; weights replicated.
  - out^T computed in [H-partition, B-free] layout; per-partition biases.
  - Mixed precision (measured end-to-end rel-err ~9.6e-3 vs the 2e-2 gate):
      * r-gate: both sides fp8e4m3 (x*16, w*512; dequant 2^-13 via the
        sigmoid's scale) in DoubleRow perf mode -> 2x PE throughput.
      * z/n gates: bf16 activations + weights on both sides.
      * final blend out = h + (1-z)*(n - h) reads a per-(b,m) fp32 h slab;
        1-z comes from one sigmoid with scale=-1.
  - b-major phasing: all 8 m-tiles of batch-half 0, then batch-half 1.
    Weights re-stream per half (DMA total ~34 MiB/core < PE span); the
    second half's activations prefetch during the first.
  - r-gate chains run one m-tile ahead of the z/n chains.
  - ~256KiB DMA granularity: big enough to stay data-bound on the HWDGE
    (~625ns/descriptor), small enough that the m=0 chains can chase the
    stream (the dep tracker is region-precise).
  - Per (m,b): n-gate chains before z-gate chains so the t/u/tanh path
    overlaps the z matmuls; the very last tile's epilogue runs in two
    column halves to pipeline sigmoid/blend/DMA in the tail.
  - reps>1 repeats the whole per-rep body (DMA + compute) for slope timing.
"""

import numpy as np

B, I, H = 8192, 1024, 1024
NCORES = 8
BL = B // NCORES  # 1024 batch rows per core
P = 128           # partitions
KT = I // P       # 8 contraction tiles
MT = H // P       # 8 output H tiles
NB = 512          # moving free dim per matmul
NBT = BL // NB    # 2 batch slices
KP = KT // 2      # 4 fp8 DoubleRow k-pairs

XS = 16.0         # fp8 activation scale
WS = 512.0        # fp8 weight scale
DESCALE = 1.0 / (XS * WS)  # 2^-13, exact

_built = {}  # reps -> nc


def _build(reps=1):
    import concourse.bass as bass
    import concourse.mybir as mybir
    from concourse.bass import ts
    from concourse.tile import TileContext

    dt = mybir.dt
    f32 = dt.float32
    bf16 = dt.bfloat16
    f8 = dt.float8e4
    ACT = mybir.ActivationFunctionType
    ALU = mybir.AluOpType
    DR = mybir.MatmulPerfMode.DoubleRow

    nc = bass.Bass()
    hT = nc.declare_dram_parameter("hT", [NBT, MT, P, NB], f32, isOutput=False)
    xbT = nc.declare_dram_parameter("xbT", [NBT, P, KT, NB], bf16, isOutput=False)
    hbT = nc.declare_dram_parameter("hbT", [NBT, P, KT, NB], bf16, isOutput=False)
    x8 = nc.declare_dram_parameter("x8", [NBT, P, KT, NB], f8, isOutput=False)
    h8 = nc.declare_dram_parameter("h8", [NBT, P, KT, NB], f8, isOutput=False)
    # wzn: per m one packed bf16 block [P, 4, KT*P] = wnx | wnh | wzx | wzh
    wzn = nc.declare_dram_parameter("wzn", [MT, P, 4, KT * P], bf16, isOutput=False)
    # wr8: per m one packed fp8 block [P, 2, KT, P] = x-side | h-side
    wr8 = nc.declare_dram_parameter("wr8", [MT, P, 2, KT, P], f8, isOutput=False)
    # bias: packed [P, 4*MT] = br | bzn | bxn | bhn
    bias = nc.declare_dram_parameter("bias", [P, 4 * MT], f32, isOutput=False)
    outT = nc.declare_dram_parameter("outT", [NBT, H, NB], f32, isOutput=True)

    with TileContext(nc) as tc:
        with (
            tc.tile_pool(name="const", bufs=1) as cpool,
            tc.tile_pool(name="acts", bufs=2) as apool,
            tc.tile_pool(name="w", bufs=2) as wpool,
            tc.tile_pool(name="ew", bufs=2) as epool,
            tc.tile_pool(name="ob", bufs=3) as opool,
            tc.tile_pool(name="ps", bufs=2, space="PSUM") as ppool,
        ):
            bias_t = cpool.tile([P, 4 * MT], f32, tag="bias")

            def bcol(g, m):  # per-partition bias column
                return bias_t[:, g * MT + m: g * MT + m + 1]

            for _rep in range(reps):
                for b in range(NBT):
                    # ---- DMA stream for this b-phase, in consumption order.
                    def wr_tile(m):
                        wt = wpool.tile([P, 2, KT, P], f8, tag="wr", name="wr")
                        nc.sync.dma_start(out=wt[:], in_=wr8[m])
                        return wt

                    wr_m = wr_tile(0)
                    x8t = apool.tile([P, KT, NB], f8, tag="x8")
                    h8t = apool.tile([P, KT, NB], f8, tag="h8")
                    for c in range(2):
                        cs = slice(4 * c, 4 * c + 4)
                        nc.sync.dma_start(out=x8t[:, cs, :], in_=x8[b][:, cs, :])
                        nc.sync.dma_start(out=h8t[:, cs, :], in_=h8[b][:, cs, :])
                    if _rep == 0 and b == 0:
                        nc.sync.dma_start(out=bias_t[:], in_=bias[:])

                    def zn_tile(m):
                        wt = wpool.tile([P, 4, KT * P], bf16, tag="wzn", name="wzn")
                        nc.sync.dma_start(out=wt[:, 0:2, :], in_=wzn[m][:, 0:2, :])
                        return wt

                    def z_part(m, wt):
                        nc.sync.dma_start(out=wt[:, 2:4, :], in_=wzn[m][:, 2:4, :])

                    def hf_tile(m):
                        hf = wpool.tile([P, NB], f32, tag="hf", name="hf")
                        nc.sync.dma_start(out=hf[:], in_=hT[b, m])
                        return hf

                    wzn_m = zn_tile(0)
                    xbt = apool.tile([P, KT, NB], bf16, tag="xbt")
                    hbt = apool.tile([P, KT, NB], bf16, tag="hbt")
                    for c in range(KP):
                        cs = slice(2 * c, 2 * c + 2)
                        nc.sync.dma_start(out=xbt[:, cs, :], in_=xbT[b][:, cs, :])
                        nc.sync.dma_start(out=hbt[:, cs, :], in_=hbT[b][:, cs, :])
                    z_part(0, wzn_m)
                    hf_m = hf_tile(0)

                    rcache = {}

                    def do_r(m, wt):
                        nc.tensor.ldweights(wt.bitcast(bf16)[:, 0, 0, 0:1])
                        s_r = ppool.tile([P, NB], f32, tag="s_r", name="s_r")
                        for kk in range(KP):
                            sl = slice(2 * kk, 2 * kk + 2)
                            nc.tensor.matmul(
                                s_r[:], wt[:, 0, sl, :], x8t[:, sl, :],
                                start=kk == 0, stop=False, perf_mode=DR)
                            nc.tensor.matmul(
                                s_r[:], wt[:, 1, sl, :], h8t[:, sl, :],
                                start=False, stop=kk == KP - 1, perf_mode=DR)
                        r = epool.tile([P, NB], f32, tag="r", name="r")
                        nc.scalar.activation(
                            r[:], s_r[:], ACT.Sigmoid,
                            bias=bcol(0, m), scale=DESCALE)
                        rcache[m] = r

                    do_r(0, wr_m)
                    for m in range(MT):
                        w, hf = wzn_m, hf_m
                        if m + 1 < MT:
                            # prefetch next m: r-gate weights + chain, then
                            # the zn weights + blend slab.
                            wr_n = wr_tile(m + 1)
                            do_r(m + 1, wr_n)
                            wzn_m = zn_tile(m + 1)
                            z_part(m + 1, wzn_m)
                            hf_m = hf_tile(m + 1)

                        ps = {
                            "s_z": ppool.tile([P, NB], f32, tag="s_z", name="s_z"),
                            "g2x": ppool.tile([P, NB], f32, tag="g2x", name="g2x"),
                            "g2h": ppool.tile([P, NB], f32, tag="g2h", name="g2h"),
                        }
                        nc.tensor.ldweights(w.bitcast(bf16)[:, 0, 0:1])
                        # n-gate x/h interleaved per k (matches slab arrival
                        # order on m=0), then the z-gate.
                        for k in range(KT):
                            first, last = k == 0, k == KT - 1
                            nc.tensor.matmul(
                                ps["g2x"][:], w[:, 0, ts(k, P)], xbt[:, k, :],
                                start=first, stop=last)
                            nc.tensor.matmul(
                                ps["g2h"][:], w[:, 1, ts(k, P)], hbt[:, k, :],
                                start=first, stop=last)
                        for k in range(KT):
                            first, last = k == 0, k == KT - 1
                            nc.tensor.matmul(
                                ps["s_z"][:], w[:, 2, ts(k, P)], xbt[:, k, :],
                                start=first, stop=False)
                            nc.tensor.matmul(
                                ps["s_z"][:], w[:, 3, ts(k, P)], hbt[:, k, :],
                                start=False, stop=last)

                        r = rcache.pop(m)
                        t = epool.tile([P, NB], f32, tag="t", name="t")
                        u = epool.tile([P, NB], f32, tag="u", name="u")
                        n = epool.tile([P, NB], f32, tag="n", name="n")
                        zc = epool.tile([P, NB], f32, tag="zc", name="zc")
                        d = epool.tile([P, NB], f32, tag="d", name="d")
                        e = epool.tile([P, NB], f32, tag="e", name="e")
                        ob = opool.tile([P, NB], f32, tag="ob", name="ob")
                        # t = (g2h + bhn) * r ; u = (g2x + bxn) + t
                        nc.vector.scalar_tensor_tensor(
                            t[:], ps["g2h"][:], bcol(3, m), r[:],
                            op0=ALU.add, op1=ALU.mult)
                        nc.vector.scalar_tensor_tensor(
                            u[:], ps["g2x"][:], bcol(2, m), t[:],
                            op0=ALU.add, op1=ALU.add)
                        nc.scalar.activation(n[:], u[:], ACT.Tanh)
                        # d = n - h  (ready before the z chain finishes)
                        nc.vector.tensor_sub(d[:], n[:], hf[:])
                        # zc = 1 - z = sigmoid(-(s_z + bz)); out = h + zc*d.
                        # Last tile of the run: two column halves to pipeline
                        # ACT -> DVE -> DVE -> DMA in the tail.
                        halves = (
                            (slice(0, NB // 2), slice(NB // 2, NB))
                            if (b == NBT - 1 and m == MT - 1) else (slice(0, NB),)
                        )
                        for hs in halves:
                            nc.scalar.activation(
                                zc[:, hs], ps["s_z"][:, hs], ACT.Sigmoid,
                                bias=bcol(1, m), scale=-1.0)
                            nc.vector.tensor_mul(e[:, hs], d[:, hs], zc[:, hs])
                            nc.vector.tensor_add(ob[:, hs], e[:, hs], hf[:, hs])
                            nc.sync.dma_start(
                                out=outT[b, ts(m, P), hs], in_=ob[:, hs])

    _split_waits(nc)
    return nc


def _split_waits(nc):
    """Walrus codegen encodes at most one semaphore wait per engine
    instruction. Tile can emit several; split the extras onto InstNoOp
    instructions inserted immediately before (same engine, same order --
    semantically identical to the multi-wait)."""
    import concourse.mybir as mybir

    SKIP = ("InstEventSemaphore", "InstCall", "InstUnconditionalBranch")
    for bb in nc.main_func.blocks:
        insts = list(bb.instructions)
        out = []
        changed = False
        for inst in insts:
            si = inst.sync_info
            nm = type(inst).__name__
            if (si is not None and si.on_wait and len(si.on_wait) > 1
                    and nm not in SKIP):
                waits = list(si.on_wait)
                for w in waits[:-1]:
                    nop = mybir.InstNoOp(
                        name=nc.get_next_instruction_name(),
                        engine=inst.engine, ins=[], outs=[])
                    nop.sync_info = mybir.SyncInfo(on_wait=[w], on_update=[])
                    nc.register_instruction(nop)
                    out.append(nop)
                inst.sync_info = mybir.SyncInfo(
                    on_wait=[waits[-1]], on_update=list(si.on_update or []))
                changed = True
            out.append(inst)
        if changed:
            bb.instructions = out


def _tile_w(w, np_dtype, scale=1.0):
    # [I, H] -> [MT, P, KT*P]: out[m, p, k*128+j] = w[k*128+p, m*128+j]
    w5 = np.asarray(w, np.float32) * scale
    w5 = w5.reshape(KT, P, MT, P).transpose(2, 1, 0, 3)  # m, p, k, j
    return np.ascontiguousarray(w5.reshape(MT, P, KT * P).astype(np_dtype))


def _tile_b(vec):  # [H] -> [P, MT] with [p, m] = vec[m*128+p]
    return np.ascontiguousarray(np.asarray(vec, np.float32).reshape(MT, P).T)


def _prep_shared(wx, wh, bx, bh):
    import ml_dtypes

    bf16 = ml_dtypes.bfloat16
    f8 = ml_dtypes.float8_e4m3
    wx = np.asarray(wx, np.float32)
    wh = np.asarray(wh, np.float32)
    bx = np.asarray(bx, np.float32)
    bh = np.asarray(bh, np.float32)

    # [MT, P, 2, KT, P]
    wr8 = np.stack([
        _tile_w(wx[0], f8, WS).reshape(MT, P, KT, P),
        _tile_w(wh[0], f8, WS).reshape(MT, P, KT, P),
    ], axis=2)
    # [MT, P, 4, KT*P] = wnx | wnh | wzx | wzh
    wzn = np.stack([
        _tile_w(wx[2], bf16),
        _tile_w(wh[2], bf16),
        _tile_w(wx[1], bf16),
        _tile_w(wh[1], bf16),
    ], axis=2)
    # [P, 4*MT] = br | bzn | bxn | bhn
    bias = np.concatenate([
        _tile_b(bx[0] + bh[0]),
        _tile_b(-(bx[1] + bh[1])),
        _tile_b(bx[2]),
        _tile_b(bh[2]),
    ], axis=1)
    return {
        "wzn": np.ascontiguousarray(wzn),
        "wr8": np.ascontiguousarray(wr8),
        "bias": np.ascontiguousarray(bias),
    }


def _prep_core(x_rows, h_rows):
    """Per-core activation tensors from [BL, I]/[BL, H] fp32 row shards."""
    import ml_dtypes

    bf16 = ml_dtypes.bfloat16
    f8 = ml_dtypes.float8_e4m3
    xT = np.ascontiguousarray(x_rows.T)                 # [I, BL] f32
    hT = np.ascontiguousarray(h_rows.T)                 # [H, BL] f32

    def quad(aT, np_dtype, scale=1.0):
        # [D, BL] -> [NBT, P, KT, NB]
        a = (aT * scale).astype(np_dtype).reshape(KT, P, NBT, NB)
        return np.ascontiguousarray(a.transpose(2, 1, 0, 3))

    # hT blend slabs: [NBT, MT, P, NB]
    hfs = hT.reshape(MT, P, NBT, NB).transpose(2, 0, 1, 3)
    return {
        "hT": np.ascontiguousarray(hfs),
        "xbT": quad(xT, bf16),
        "hbT": quad(hT, bf16),
        "x8": quad(xT, f8, XS),
        "h8": quad(hT, f8, XS),
    }


def make_in_maps(x, hid, wx, wh, bx, bh):
    x = np.asarray(x, np.float32)
    hid = np.asarray(hid, np.float32)
    shared = _prep_shared(wx, wh, bx, bh)
    in_maps = []
    for c in range(NCORES):
        rows = slice(c * BL, (c + 1) * BL)
        m = dict(shared)
        m.update(_prep_core(x[rows], hid[rows]))
        in_maps.append(m)
    return in_maps


def kernel(x, hid, wx, wh, bx, bh):
    from concourse.bass_utils import run_bass_kernel_spmd

    if 1 not in _built:
        _built[1] = _build(reps=1)
    nc = _built[1]

    in_maps = make_in_maps(x, hid, wx, wh, bx, bh)
    res = run_bass_kernel_spmd(nc, in_maps, list(range(NCORES)))
    out = np.empty((B, H), np.float32)
    for c in range(NCORES):
        o = res.results[c]["outT"]  # [NBT, H, NB]
        for b in range(NBT):
            out[c * BL + b * NB: c * BL + (b + 1) * NB] = o[b].T
    return out


# revision 12
# speedup vs baseline: 1.1804x; 1.1780x over previous
"""GRU cell kernel for Trainium2, data-parallel over 8 NeuronCores.

Computation (per reference):
    gx[g] = x @ wx[g] + bx[g]
    gh[g] = hid @ wh[g] + bh[g]
    r = sigmoid(gx0 + gh0); z = sigmoid(gx1 + gh1)
    n = tanh(gx2 + r * gh2)
    out = (1 - z) * n + z * hid

Design (v6):
  - Batch (8192) sharded 8 ways -> 1024 rows/core; weights replicated.
  - out^T computed in [H-partition, B-free] layout; per-partition biases.
  - Mixed precision (measured end-to-end rel-err ~9.6e-3 vs the 2e-2 gate):
      * r-gate: both sides fp8e4m3 (x*16, w*512; dequant 2^-13 via the
        sigmoid's scale) in DoubleRow perf mode -> 2x PE throughput.
      * z/n gates: bf16 activations + weights on both sides.
      * final blend out = h + (1-z)*(n - h) reads a per-(b,m) fp32 h slab;
        1-z comes from one sigmoid with scale=-1.
  - b-major phasing: all 8 m-tiles of batch-half 0, then batch-half 1.
    Weights re-stream per half (DMA total ~34 MiB/core < PE span); the
    second half's activations prefetch during the first.
  - r-gate chains run one m-tile ahead of the z/n chains.
  - ~256KiB DMA granularity: big enough to stay data-bound on the HWDGE
    (~625ns/descriptor), small enough that the m=0 chains can chase the
    stream (the dep tracker is region-precise).
  - Per (m,b): n-gate chains before z-gate chains so the t/u/tanh path
    overlaps the z matmuls; the very last tile's epilogue runs in two
    column halves to pipeline sigmoid/blend/DMA in the tail.
  - reps>1 repeats the whole per-rep body (DMA + compute) for slope timing.
"""

import numpy as np

B, I, H = 8192, 1024, 1024
NCORES = 8
BL = B // NCORES  # 1024 batch rows per core
P = 128           # partitions
KT = I // P       # 8 contraction tiles
MT = H // P       # 8 output H tiles
NB = 512          # moving free dim per matmul
NBT = BL // NB    # 2 batch slices
KP = KT // 2      # 4 fp8 DoubleRow k-pairs

XS = 16.0         # fp8 activation scale
WS = 512.0        # fp8 weight scale
DESCALE = 1.0 / (XS * WS)  # 2^-13, exact

_built = {}  # reps -> nc


def _build(reps=1):
    import concourse.bass as bass
    import concourse.mybir as mybir
    from concourse.bass import ts
    from concourse.tile import TileContext

    dt = mybir.dt
    f32 = dt.float32
    bf16 = dt.bfloat16
    f8 = dt.float8e4
    ACT = mybir.ActivationFunctionType
    ALU = mybir.AluOpType
    DR = mybir.MatmulPerfMode.DoubleRow

    nc = bass.Bass()
    hT = nc.declare_dram_parameter("hT", [NBT, MT, P, NB], f32, isOutput=False)
    xbT = nc.declare_dram_parameter("xbT", [NBT, P, KT, NB], bf16, isOutput=False)
    hbT = nc.declare_dram_parameter("hbT", [NBT, P, KT, NB], bf16, isOutput=False)
    x8 = nc.declare_dram_parameter("x8", [NBT, P, KT, NB], f8, isOutput=False)
    h8 = nc.declare_dram_parameter("h8", [NBT, P, KT, NB], f8, isOutput=False)
    # wzn: per m one packed bf16 block [P, 4, KT*P] = wnx | wnh | wzx | wzh
    wzn = nc.declare_dram_parameter("wzn", [MT, P, 4, KT * P], bf16, isOutput=False)
    # wr8: per m one packed fp8 block [P, 2, KT, P] = x-side | h-side
    wr8 = nc.declare_dram_parameter("wr8", [MT, P, 2, KT, P], f8, isOutput=False)
    # bias: packed [P, 4*MT] = br | bzn | bxn | bhn
    bias = nc.declare_dram_parameter("bias", [P, 4 * MT], f32, isOutput=False)
    outT = nc.declare_dram_parameter("outT", [NBT, H, NB], f32, isOutput=True)

    with TileContext(nc) as tc:
        with (
            tc.tile_pool(name="const", bufs=1) as cpool,
            tc.tile_pool(name="acts", bufs=2) as apool,
            tc.tile_pool(name="w", bufs=2) as wpool,
            tc.tile_pool(name="ew", bufs=2) as epool,
            tc.tile_pool(name="ob", bufs=3) as opool,
            tc.tile_pool(name="ps", bufs=2, space="PSUM") as ppool,
        ):
            bias_t = cpool.tile([P, 4 * MT], f32, tag="bias")

            def bcol(g, m):  # per-partition bias column
                return bias_t[:, g * MT + m: g * MT + m + 1]

            for _rep in range(reps):
                for b in range(NBT):
                    # ---- DMA stream for this b-phase, in consumption order.
                    def wr_tile(m):
                        wt = wpool.tile([P, 2, KT, P], f8, tag="wr", name="wr")
                        nc.sync.dma_start(out=wt[:], in_=wr8[m])
                        return wt

                    RFRONT = 1
                    wr_tiles = {0: wr_tile(0)}
                    x8t = apool.tile([P, KT, NB], f8, tag="x8")
                    h8t = apool.tile([P, KT, NB], f8, tag="h8")
                    for c in range(2):
                        cs = slice(4 * c, 4 * c + 4)
                        nc.sync.dma_start(out=x8t[:, cs, :], in_=x8[b][:, cs, :])
                        nc.sync.dma_start(out=h8t[:, cs, :], in_=h8[b][:, cs, :])
                    if _rep == 0 and b == 0:
                        nc.sync.dma_start(out=bias_t[:], in_=bias[:])
                    for mm in range(1, RFRONT):
                        wr_tiles[mm] = wr_tile(mm)

                    def zn_tile(m):
                        wt = wpool.tile([P, 4, KT * P], bf16, tag="wzn", name="wzn")
                        nc.sync.dma_start(out=wt[:, 0:2, :], in_=wzn[m][:, 0:2, :])
                        return wt

                    def z_part(m, wt):
                        nc.sync.dma_start(out=wt[:, 2:4, :], in_=wzn[m][:, 2:4, :])

                    def hf_tile(m):
                        hf = wpool.tile([P, NB], f32, tag="hf", name="hf")
                        nc.sync.dma_start(out=hf[:], in_=hT[b, m])
                        return hf

                    wzn_m = zn_tile(0)
                    xbt = apool.tile([P, KT, NB], bf16, tag="xbt")
                    hbt = apool.tile([P, KT, NB], bf16, tag="hbt")
                    for c in range(KP):
                        cs = slice(2 * c, 2 * c + 2)
                        nc.sync.dma_start(out=xbt[:, cs, :], in_=xbT[b][:, cs, :])
                        nc.sync.dma_start(out=hbt[:, cs, :], in_=hbT[b][:, cs, :])
                    z_part(0, wzn_m)
                    hf_m = hf_tile(0)

                    rcache = {}

                    def do_r(m, wt):
                        nc.tensor.ldweights(wt.bitcast(bf16)[:, 0, 0, 0:1])
                        s_r = ppool.tile([P, NB], f32, tag="s_r", name="s_r")
                        for kk in range(KP):
                            sl = slice(2 * kk, 2 * kk + 2)
                            nc.tensor.matmul(
                                s_r[:], wt[:, 0, sl, :], x8t[:, sl, :],
                                start=kk == 0, stop=False, perf_mode=DR)
                            nc.tensor.matmul(
                                s_r[:], wt[:, 1, sl, :], h8t[:, sl, :],
                                start=False, stop=kk == KP - 1, perf_mode=DR)
                        r = epool.tile([P, NB], f32, tag="r", name="r", bufs=RFRONT + 1)
                        nc.scalar.activation(
                            r[:], s_r[:], ACT.Sigmoid,
                            bias=bcol(0, m), scale=DESCALE)
                        rcache[m] = r

                    for mm in range(RFRONT):
                        do_r(mm, wr_tiles.pop(mm))
                    for m in range(MT):
                        w, hf = wzn_m, hf_m
                        if m + RFRONT < MT:
                            do_r(m + RFRONT, wr_tile(m + RFRONT))
                        if m + 1 < MT:
                            # prefetch next m's zn weights + blend slab.
                            wzn_m = zn_tile(m + 1)
                            z_part(m + 1, wzn_m)
                            hf_m = hf_tile(m + 1)

                        ps = {
                            "s_z": ppool.tile([P, NB], f32, tag="s_z", name="s_z"),
                            "g2x": ppool.tile([P, NB], f32, tag="g2x", name="g2x"),
                            "g2h": ppool.tile([P, NB], f32, tag="g2h", name="g2h"),
                        }
                        nc.tensor.ldweights(w.bitcast(bf16)[:, 0, 0:1])
                        # n-gate x/h interleaved per k (matches slab arrival
                        # order on m=0), then the z-gate.
                        for k in range(KT):
                            first, last = k == 0, k == KT - 1
                            nc.tensor.matmul(
                                ps["g2x"][:], w[:, 0, ts(k, P)], xbt[:, k, :],
                                start=first, stop=last)
                            nc.tensor.matmul(
                                ps["g2h"][:], w[:, 1, ts(k, P)], hbt[:, k, :],
                                start=first, stop=last)
                        for k in range(KT):
                            first, last = k == 0, k == KT - 1
                            nc.tensor.matmul(
                                ps["s_z"][:], w[:, 2, ts(k, P)], xbt[:, k, :],
                                start=first, stop=False)
                            nc.tensor.matmul(
                                ps["s_z"][:], w[:, 3, ts(k, P)], hbt[:, k, :],
                                start=False, stop=last)

                        r = rcache.pop(m)
                        t = epool.tile([P, NB], f32, tag="t", name="t")
                        u = epool.tile([P, NB], f32, tag="u", name="u")
                        n = epool.tile([P, NB], f32, tag="n", name="n")
                        zc = epool.tile([P, NB], f32, tag="zc", name="zc")
                        d = epool.tile([P, NB], f32, tag="d", name="d")
                        e = epool.tile([P, NB], f32, tag="e", name="e")
                        ob = opool.tile([P, NB], f32, tag="ob", name="ob")
                        # t = (g2h + bhn) * r ; u = (g2x + bxn) + t
                        nc.vector.scalar_tensor_tensor(
                            t[:], ps["g2h"][:], bcol(3, m), r[:],
                            op0=ALU.add, op1=ALU.mult)
                        nc.vector.scalar_tensor_tensor(
                            u[:], ps["g2x"][:], bcol(2, m), t[:],
                            op0=ALU.add, op1=ALU.add)
                        nc.scalar.activation(n[:], u[:], ACT.Tanh)
                        # d = n - h  (ready before the z chain finishes)
                        nc.vector.tensor_sub(d[:], n[:], hf[:])
                        # zc = 1 - z = sigmoid(-(s_z + bz)); out = h + zc*d.
                        # Last tile of the run: two column halves to pipeline
                        # ACT -> DVE -> DVE -> DMA in the tail.
                        halves = (
                            (slice(0, NB // 2), slice(NB // 2, NB))
                            if (b == NBT - 1 and m == MT - 1) else (slice(0, NB),)
                        )
                        for hs in halves:
                            nc.scalar.activation(
                                zc[:, hs], ps["s_z"][:, hs], ACT.Sigmoid,
                                bias=bcol(1, m), scale=-1.0)
                            nc.gpsimd.tensor_mul(e[:, hs], d[:, hs], zc[:, hs])
                            nc.vector.tensor_add(ob[:, hs], e[:, hs], hf[:, hs])
                            nc.sync.dma_start(
                                out=outT[b, ts(m, P), hs], in_=ob[:, hs])

    _split_waits(nc)
    return nc


def _split_waits(nc):
    """Walrus codegen encodes at most one semaphore wait per engine
    instruction. Tile can emit several; split the extras onto InstNoOp
    instructions inserted immediately before (same engine, same order --
    semantically identical to the multi-wait)."""
    import concourse.mybir as mybir

    SKIP = ("InstEventSemaphore", "InstCall", "InstUnconditionalBranch")
    for bb in nc.main_func.blocks:
        insts = list(bb.instructions)
        out = []
        changed = False
        for inst in insts:
            si = inst.sync_info
            nm = type(inst).__name__
            if (si is not None and si.on_wait and len(si.on_wait) > 1
                    and nm not in SKIP):
                waits = list(si.on_wait)
                for w in waits[:-1]:
                    nop = mybir.InstNoOp(
                        name=nc.get_next_instruction_name(),
                        engine=inst.engine, ins=[], outs=[])
                    nop.sync_info = mybir.SyncInfo(on_wait=[w], on_update=[])
                    nc.register_instruction(nop)
                    out.append(nop)
                inst.sync_info = mybir.SyncInfo(
                    on_wait=[waits[-1]], on_update=list(si.on_update or []))
                changed = True
            out.append(inst)
        if changed:
            bb.instructions = out


def _tile_w(w, np_dtype, scale=1.0):
    # [I, H] -> [MT, P, KT*P]: out[m, p, k*128+j] = w[k*128+p, m*128+j]
    w5 = np.asarray(w, np.float32) * scale
    w5 = w5.reshape(KT, P, MT, P).transpose(2, 1, 0, 3)  # m, p, k, j
    return np.ascontiguousarray(w5.reshape(MT, P, KT * P).astype(np_dtype))


def _tile_b(vec):  # [H] -> [P, MT] with [p, m] = vec[m*128+p]
    return np.ascontiguousarray(np.asarray(vec, np.float32).reshape(MT, P).T)


def _prep_shared(wx, wh, bx, bh):
    import ml_dtypes

    bf16 = ml_dtypes.bfloat16
    f8 = ml_dtypes.float8_e4m3
    wx = np.asarray(wx, np.float32)
    wh = np.asarray(wh, np.float32)
    bx = np.asarray(bx, np.float32)
    bh = np.asarray(bh, np.float32)

    # [MT, P, 2, KT, P]
    wr8 = np.stack([
        _tile_w(wx[0], f8, WS).reshape(MT, P, KT, P),
        _tile_w(wh[0], f8, WS).reshape(MT, P, KT, P),
    ], axis=2)
    # [MT, P, 4, KT*P] = wnx | wnh | wzx | wzh
    wzn = np.stack([
        _tile_w(wx[2], bf16),
        _tile_w(wh[2], bf16),
        _tile_w(wx[1], bf16),
        _tile_w(wh[1], bf16),
    ], axis=2)
    # [P, 4*MT] = br | bzn | bxn | bhn
    bias = np.concatenate([
        _tile_b(bx[0] + bh[0]),
        _tile_b(-(bx[1] + bh[1])),
        _tile_b(bx[2]),
        _tile_b(bh[2]),
    ], axis=1)
    return {
        "wzn": np.ascontiguousarray(wzn),
        "wr8": np.ascontiguousarray(wr8),
        "bias": np.ascontiguousarray(bias),
    }


def _prep_core(x_rows, h_rows):
    """Per-core activation tensors from [BL, I]/[BL, H] fp32 row shards."""
    import ml_dtypes

    bf16 = ml_dtypes.bfloat16
    f8 = ml_dtypes.float8_e4m3
    xT = np.ascontiguousarray(x_rows.T)                 # [I, BL] f32
    hT = np.ascontiguousarray(h_rows.T)                 # [H, BL] f32

    def quad(aT, np_dtype, scale=1.0):
        # [D, BL] -> [NBT, P, KT, NB]
        a = (aT * scale).astype(np_dtype).reshape(KT, P, NBT, NB)
        return np.ascontiguousarray(a.transpose(2, 1, 0, 3))

    # hT blend slabs: [NBT, MT, P, NB]
    hfs = hT.reshape(MT, P, NBT, NB).transpose(2, 0, 1, 3)
    return {
        "hT": np.ascontiguousarray(hfs),
        "xbT": quad(xT, bf16),
        "hbT": quad(hT, bf16),
        "x8": quad(xT, f8, XS),
        "h8": quad(hT, f8, XS),
    }


def make_in_maps(x, hid, wx, wh, bx, bh):
    x = np.asarray(x, np.float32)
    hid = np.asarray(hid, np.float32)
    shared = _prep_shared(wx, wh, bx, bh)
    in_maps = []
    for c in range(NCORES):
        rows = slice(c * BL, (c + 1) * BL)
        m = dict(shared)
        m.update(_prep_core(x[rows], hid[rows]))
        in_maps.append(m)
    return in_maps


def kernel(x, hid, wx, wh, bx, bh):
    from concourse.bass_utils import run_bass_kernel_spmd

    if 1 not in _built:
        _built[1] = _build(reps=1)
    nc = _built[1]

    in_maps = make_in_maps(x, hid, wx, wh, bx, bh)
    res = run_bass_kernel_spmd(nc, in_maps, list(range(NCORES)))
    out = np.empty((B, H), np.float32)
    for c in range(NCORES):
        o = res.results[c]["outT"]  # [NBT, H, NB]
        for b in range(NBT):
            out[c * BL + b * NB: c * BL + (b + 1) * NB] = o[b].T
    return out


# revision 13
# speedup vs baseline: 1.3661x; 1.1573x over previous
"""GRU cell kernel for Trainium2, data-parallel over 8 NeuronCores.

Computation (per reference):
    gx[g] = x @ wx[g] + bx[g]
    gh[g] = hid @ wh[g] + bh[g]
    r = sigmoid(gx0 + gh0); z = sigmoid(gx1 + gh1)
    n = tanh(gx2 + r * gh2)
    out = (1 - z) * n + z * hid

Design (v6):
  - Batch (8192) sharded 8 ways -> 1024 rows/core; weights replicated.
  - out^T computed in [H-partition, B-free] layout; per-partition biases.
  - Mixed precision (measured end-to-end rel-err ~9.6e-3 vs the 2e-2 gate):
      * r-gate: both sides fp8e4m3 (x*16, w*512; dequant 2^-13 via the
        sigmoid's scale) in DoubleRow perf mode -> 2x PE throughput.
      * z/n gates: bf16 activations + weights on both sides.
      * final blend out = h + (1-z)*(n - h) reads a per-(b,m) fp32 h slab;
        1-z comes from one sigmoid with scale=-1.
  - b-major phasing: all 8 m-tiles of batch-half 0, then batch-half 1.
    Weights re-stream per half (DMA total ~34 MiB/core < PE span); the
    second half's activations prefetch during the first.
  - r-gate chains run one m-tile ahead of the z/n chains.
  - ~256KiB DMA granularity: big enough to stay data-bound on the HWDGE
    (~625ns/descriptor), small enough that the m=0 chains can chase the
    stream (the dep tracker is region-precise).
  - Per (m,b): n-gate chains before z-gate chains so the t/u/tanh path
    overlaps the z matmuls; the very last tile's epilogue runs in two
    column halves to pipeline sigmoid/blend/DMA in the tail.
  - reps>1 repeats the whole per-rep body (DMA + compute) for slope timing.
"""

import numpy as np

B, I, H = 8192, 1024, 1024
NCORES = 8
BL = B // NCORES  # 1024 batch rows per core
P = 128           # partitions
KT = I // P       # 8 contraction tiles
MT = H // P       # 8 output H tiles
NB = 512          # moving free dim per matmul
NBT = BL // NB    # 2 batch slices
KP = KT // 2      # 4 fp8 DoubleRow k-pairs

XS = 16.0         # fp8 activation scale
WS = 512.0        # fp8 weight scale
DESCALE = 1.0 / (XS * WS)  # 2^-13, exact

_built = {}  # reps -> nc


def _build(reps=1):
    import concourse.bass as bass
    import concourse.mybir as mybir
    from concourse.bass import ts
    from concourse.tile import TileContext

    dt = mybir.dt
    f32 = dt.float32
    bf16 = dt.bfloat16
    f8 = dt.float8e4
    ACT = mybir.ActivationFunctionType
    ALU = mybir.AluOpType
    DR = mybir.MatmulPerfMode.DoubleRow

    nc = bass.Bass()
    hT = nc.declare_dram_parameter("hT", [NBT, MT, P, NB], f32, isOutput=False)
    xbT = nc.declare_dram_parameter("xbT", [NBT, P, KT, NB], bf16, isOutput=False)
    hbT = nc.declare_dram_parameter("hbT", [NBT, P, KT, NB], bf16, isOutput=False)
    x8 = nc.declare_dram_parameter("x8", [NBT, P, KT, NB], f8, isOutput=False)
    h8 = nc.declare_dram_parameter("h8", [NBT, P, KT, NB], f8, isOutput=False)
    # wzn: per m one packed bf16 block [P, 4, KT*P] = wnx | wnh | wzx | wzh
    wzn = nc.declare_dram_parameter("wzn", [MT, P, 4, KT * P], bf16, isOutput=False)
    # wr8: per m one packed fp8 block [P, 2, KT, P] = x-side | h-side
    wr8 = nc.declare_dram_parameter("wr8", [MT, P, 2, KT, P], f8, isOutput=False)
    # bias: packed [P, 4*MT] = br | bzn | bxn | bhn
    bias = nc.declare_dram_parameter("bias", [P, 4 * MT], f32, isOutput=False)
    outT = nc.declare_dram_parameter("outT", [NBT, H, NB], f32, isOutput=True)

    with TileContext(nc) as tc:
        with (
            tc.tile_pool(name="const", bufs=1) as cpool,
            tc.tile_pool(name="acts", bufs=2) as apool,
            tc.tile_pool(name="w", bufs=2) as wpool,
            tc.tile_pool(name="ew", bufs=2) as epool,
            tc.tile_pool(name="ob", bufs=3) as opool,
            tc.tile_pool(name="ps", bufs=2, space="PSUM") as ppool,
        ):
            bias_t = cpool.tile([P, 4 * MT], f32, tag="bias")

            def bcol(g, m):  # per-partition bias column
                return bias_t[:, g * MT + m: g * MT + m + 1]

            for _rep in range(reps):
                # Weights are loaded ONCE per rep (resident across both
                # b-phases): ~10 MiB SBUF, saving ~12 MiB of DMA re-stream.
                wr_res = {}
                wzn_res = {}

                def wr_tile(m):
                    wt = wpool.tile([P, 2, KT, P], f8, tag=f"wr{m}",
                                    name=f"wr{m}", bufs=1)
                    nc.sync.dma_start(out=wt[:], in_=wr8[m])
                    wr_res[m] = wt

                def zn_wn(m):
                    wt = wpool.tile([P, 4, KT * P], bf16, tag=f"wzn{m}",
                                    name=f"wzn{m}", bufs=1)
                    nc.sync.dma_start(out=wt[:, 0:2, :], in_=wzn[m][:, 0:2, :])
                    wzn_res[m] = wt

                def zn_wz(m):
                    nc.sync.dma_start(
                        out=wzn_res[m][:, 2:4, :], in_=wzn[m][:, 2:4, :])

                for b in range(NBT):
                    first_b = b == 0
                    # ---- DMA stream for this b-phase, in consumption order.
                    if first_b:
                        wr_tile(0)
                    x8t = apool.tile([P, KT, NB], f8, tag="x8")
                    h8t = apool.tile([P, KT, NB], f8, tag="h8")
                    for c in range(2):
                        cs = slice(4 * c, 4 * c + 4)
                        nc.sync.dma_start(out=x8t[:, cs, :], in_=x8[b][:, cs, :])
                        nc.sync.dma_start(out=h8t[:, cs, :], in_=h8[b][:, cs, :])
                    if _rep == 0 and first_b:
                        nc.sync.dma_start(out=bias_t[:], in_=bias[:])

                    def hf_tile(m):
                        hf = wpool.tile([P, NB], f32, tag="hf", name="hf")
                        nc.sync.dma_start(out=hf[:], in_=hT[b, m])
                        return hf

                    if first_b:
                        zn_wn(0)
                    xbt = apool.tile([P, KT, NB], bf16, tag="xbt")
                    hbt = apool.tile([P, KT, NB], bf16, tag="hbt")
                    for c in range(KP):
                        cs = slice(2 * c, 2 * c + 2)
                        nc.sync.dma_start(out=xbt[:, cs, :], in_=xbT[b][:, cs, :])
                        nc.sync.dma_start(out=hbt[:, cs, :], in_=hbT[b][:, cs, :])
                    if first_b:
                        zn_wz(0)
                    hf_m = hf_tile(0)

                    rcache = {}

                    def do_r(m):
                        wt = wr_res[m]
                        nc.tensor.ldweights(wt.bitcast(bf16)[:, 0, 0, 0:1])
                        s_r = ppool.tile([P, NB], f32, tag="s_r", name="s_r")
                        for kk in range(KP):
                            sl = slice(2 * kk, 2 * kk + 2)
                            nc.tensor.matmul(
                                s_r[:], wt[:, 0, sl, :], x8t[:, sl, :],
                                start=kk == 0, stop=False, perf_mode=DR)
                            nc.tensor.matmul(
                                s_r[:], wt[:, 1, sl, :], h8t[:, sl, :],
                                start=False, stop=kk == KP - 1, perf_mode=DR)
                        r = epool.tile([P, NB], f32, tag="r", name="r")
                        nc.scalar.activation(
                            r[:], s_r[:], ACT.Sigmoid,
                            bias=bcol(0, m), scale=DESCALE)
                        rcache[m] = r

                    do_r(0)
                    for m in range(MT):
                        w, hf = wzn_res[m], hf_m
                        if m + 1 < MT:
                            if first_b:
                                wr_tile(m + 1)
                            do_r(m + 1)
                            if first_b:
                                # prefetch next m's zn weights
                                zn_wn(m + 1)
                                zn_wz(m + 1)
                            hf_m = hf_tile(m + 1)

                        ps = {
                            "s_z": ppool.tile([P, NB], f32, tag="s_z", name="s_z"),
                            "g2x": ppool.tile([P, NB], f32, tag="g2x", name="g2x"),
                            "g2h": ppool.tile([P, NB], f32, tag="g2h", name="g2h"),
                        }
                        nc.tensor.ldweights(w.bitcast(bf16)[:, 0, 0:1])
                        # n-gate x/h interleaved per k (matches slab arrival
                        # order on m=0), then the z-gate.
                        for k in range(KT):
                            first, last = k == 0, k == KT - 1
                            nc.tensor.matmul(
                                ps["g2x"][:], w[:, 0, ts(k, P)], xbt[:, k, :],
                                start=first, stop=last)
                            nc.tensor.matmul(
                                ps["g2h"][:], w[:, 1, ts(k, P)], hbt[:, k, :],
                                start=first, stop=last)
                        for k in range(KT):
                            first, last = k == 0, k == KT - 1
                            nc.tensor.matmul(
                                ps["s_z"][:], w[:, 2, ts(k, P)], xbt[:, k, :],
                                start=first, stop=False)
                            nc.tensor.matmul(
                                ps["s_z"][:], w[:, 3, ts(k, P)], hbt[:, k, :],
                                start=False, stop=last)

                        r = rcache.pop(m)
                        t = epool.tile([P, NB], f32, tag="t", name="t")
                        u = epool.tile([P, NB], f32, tag="u", name="u")
                        n = epool.tile([P, NB], f32, tag="n", name="n")
                        zc = epool.tile([P, NB], f32, tag="zc", name="zc")
                        d = epool.tile([P, NB], f32, tag="d", name="d")
                        e = epool.tile([P, NB], f32, tag="e", name="e")
                        ob = opool.tile([P, NB], f32, tag="ob", name="ob")
                        # t = (g2h + bhn) * r ; u = (g2x + bxn) + t
                        nc.vector.scalar_tensor_tensor(
                            t[:], ps["g2h"][:], bcol(3, m), r[:],
                            op0=ALU.add, op1=ALU.mult)
                        nc.vector.scalar_tensor_tensor(
                            u[:], ps["g2x"][:], bcol(2, m), t[:],
                            op0=ALU.add, op1=ALU.add)
                        nc.scalar.activation(n[:], u[:], ACT.Tanh)
                        # d = n - h  (ready before the z chain finishes)
                        nc.vector.tensor_sub(d[:], n[:], hf[:])
                        # zc = 1 - z = sigmoid(-(s_z + bz)); out = h + zc*d.
                        # Last tile of the run: two column halves to pipeline
                        # ACT -> DVE -> DVE -> DMA in the tail.
                        halves = (
                            (slice(0, NB // 2), slice(NB // 2, NB))
                            if (b == NBT - 1 and m == MT - 1) else (slice(0, NB),)
                        )
                        for hs in halves:
                            nc.scalar.activation(
                                zc[:, hs], ps["s_z"][:, hs], ACT.Sigmoid,
                                bias=bcol(1, m), scale=-1.0)
                            nc.gpsimd.tensor_mul(e[:, hs], d[:, hs], zc[:, hs])
                            nc.vector.tensor_add(ob[:, hs], e[:, hs], hf[:, hs])
                            nc.sync.dma_start(
                                out=outT[b, ts(m, P), hs], in_=ob[:, hs])

    _split_waits(nc)
    return nc


def _split_waits(nc):
    """Walrus codegen encodes at most one semaphore wait per engine
    instruction. Tile can emit several; split the extras onto InstNoOp
    instructions inserted immediately before (same engine, same order --
    semantically identical to the multi-wait)."""
    import concourse.mybir as mybir

    SKIP = ("InstEventSemaphore", "InstCall", "InstUnconditionalBranch")
    for bb in nc.main_func.blocks:
        insts = list(bb.instructions)
        out = []
        changed = False
        for inst in insts:
            si = inst.sync_info
            nm = type(inst).__name__
            if (si is not None and si.on_wait and len(si.on_wait) > 1
                    and nm not in SKIP):
                waits = list(si.on_wait)
                for w in waits[:-1]:
                    nop = mybir.InstNoOp(
                        name=nc.get_next_instruction_name(),
                        engine=inst.engine, ins=[], outs=[])
                    nop.sync_info = mybir.SyncInfo(on_wait=[w], on_update=[])
                    nc.register_instruction(nop)
                    out.append(nop)
                inst.sync_info = mybir.SyncInfo(
                    on_wait=[waits[-1]], on_update=list(si.on_update or []))
                changed = True
            out.append(inst)
        if changed:
            bb.instructions = out


def _tile_w(w, np_dtype, scale=1.0):
    # [I, H] -> [MT, P, KT*P]: out[m, p, k*128+j] = w[k*128+p, m*128+j]
    w5 = np.asarray(w, np.float32) * scale
    w5 = w5.reshape(KT, P, MT, P).transpose(2, 1, 0, 3)  # m, p, k, j
    return np.ascontiguousarray(w5.reshape(MT, P, KT * P).astype(np_dtype))


def _tile_b(vec):  # [H] -> [P, MT] with [p, m] = vec[m*128+p]
    return np.ascontiguousarray(np.asarray(vec, np.float32).reshape(MT, P).T)


def _prep_shared(wx, wh, bx, bh):
    import ml_dtypes

    bf16 = ml_dtypes.bfloat16
    f8 = ml_dtypes.float8_e4m3
    wx = np.asarray(wx, np.float32)
    wh = np.asarray(wh, np.float32)
    bx = np.asarray(bx, np.float32)
    bh = np.asarray(bh, np.float32)

    # [MT, P, 2, KT, P]
    wr8 = np.stack([
        _tile_w(wx[0], f8, WS).reshape(MT, P, KT, P),
        _tile_w(wh[0], f8, WS).reshape(MT, P, KT, P),
    ], axis=2)
    # [MT, P, 4, KT*P] = wnx | wnh | wzx | wzh
    wzn = np.stack([
        _tile_w(wx[2], bf16),
        _tile_w(wh[2], bf16),
        _tile_w(wx[1], bf16),
        _tile_w(wh[1], bf16),
    ], axis=2)
    # [P, 4*MT] = br | bzn | bxn | bhn
    bias = np.concatenate([
        _tile_b(bx[0] + bh[0]),
        _tile_b(-(bx[1] + bh[1])),
        _tile_b(bx[2]),
        _tile_b(bh[2]),
    ], axis=1)
    return {
        "wzn": np.ascontiguousarray(wzn),
        "wr8": np.ascontiguousarray(wr8),
        "bias": np.ascontiguousarray(bias),
    }


def _prep_core(x_rows, h_rows):
    """Per-core activation tensors from [BL, I]/[BL, H] fp32 row shards."""
    import ml_dtypes

    bf16 = ml_dtypes.bfloat16
    f8 = ml_dtypes.float8_e4m3
    xT = np.ascontiguousarray(x_rows.T)                 # [I, BL] f32
    hT = np.ascontiguousarray(h_rows.T)                 # [H, BL] f32

    def quad(aT, np_dtype, scale=1.0):
        # [D, BL] -> [NBT, P, KT, NB]
        a = (aT * scale).astype(np_dtype).reshape(KT, P, NBT, NB)
        return np.ascontiguousarray(a.transpose(2, 1, 0, 3))

    # hT blend slabs: [NBT, MT, P, NB]
    hfs = hT.reshape(MT, P, NBT, NB).transpose(2, 0, 1, 3)
    return {
        "hT": np.ascontiguousarray(hfs),
        "xbT": quad(xT, bf16),
        "hbT": quad(hT, bf16),
        "x8": quad(xT, f8, XS),
        "h8": quad(hT, f8, XS),
    }


def make_in_maps(x, hid, wx, wh, bx, bh):
    x = np.asarray(x, np.float32)
    hid = np.asarray(hid, np.float32)
    shared = _prep_shared(wx, wh, bx, bh)
    in_maps = []
    for c in range(NCORES):
        rows = slice(c * BL, (c + 1) * BL)
        m = dict(shared)
        m.update(_prep_core(x[rows], hid[rows]))
        in_maps.append(m)
    return in_maps


def kernel(x, hid, wx, wh, bx, bh):
    from concourse.bass_utils import run_bass_kernel_spmd

    if 1 not in _built:
        _built[1] = _build(reps=1)
    nc = _built[1]

    in_maps = make_in_maps(x, hid, wx, wh, bx, bh)
    res = run_bass_kernel_spmd(nc, in_maps, list(range(NCORES)))
    out = np.empty((B, H), np.float32)
    for c in range(NCORES):
        o = res.results[c]["outT"]  # [NBT, H, NB]
        for b in range(NBT):
            out[c * BL + b * NB: c * BL + (b + 1) * NB] = o[b].T
    return out
